# revision 13
# baseline (speedup 1.0000x reference)
"""MultiHeadAttention forward on 8 Trainium2 NeuronCores.

Sharding: core c handles batch (c // 4) and a block of 4 heads
(heads 4*(c%4) .. 4*(c%4)+4), i.e. 256 of the 1024 inner features.
Each core computes its batch's full attention for its heads plus the
partial output projection (rows of W_o for its heads); the host sums
the 4 per-batch partials (the W_o all-reduce) and adds the bias terms.

Pipeline (v2): the kernel is one software pipeline keyed on the ACT
engine (exp is the per-core floor: 4 heads x 2048 x 2048 exps).
Loop structure: q-half outer (1024 cols), then head, then kv tile.
Per (head, kv): PE scores -> ACT exp (psum fp32 -> fp16, one FD=1024
instr) -> DVE mask-mult (fp8 mask) -> PE PV accumulate (ones-column
trick for the softmax denominator).  Projections (q/k/v) and the
output projection are interleaved into the PE stream's idle slots so
the PE never idles long enough for the HAM clock gate to re-throttle.
PSUM: 2 x scores[128,1024] + 2 x pv[65,1024] = 8 banks exactly.
"""

import os

import numpy as np

B = 2
NQ = 2048
NKV = 2048
DM = 1024
H = 16
DH = 64
N_CORES = 8
CORES_PER_BATCH = N_CORES // B  # 4
HPC = H // CORES_PER_BATCH  # 4 heads per core
FPC = HPC * DH  # 256 features per core

_F16 = np.float16

_cache = {}
last_results = None  # stash of BassKernelResults for test harnesses

MASK_F8 = not int(os.environ.get("KERNEL_MASK_F16", "0"))


def _mask_np_dtype():
    if not MASK_F8:
        return np.float16
    import ml_dtypes

    return ml_dtypes.float8_e4m3


def _patch_tile_drain():
    """Split the TileContext tail-drain waits: this walrus build rejects
    Drain instructions carrying more than one sync wait."""
    import concourse.mybir as mybir
    import concourse.tile as tile
    from concourse.vector_clock import ScopedClock

    if getattr(tile.TileContext, "_drain_split_patch", False):
        return

    def _drain_and_barrier(self, tick_clock, wait_clock):
        probe = self.nc.sync.nop(nofuse=True)
        wait_clock.add_sem_waits(
            probe.ins, ScopedClock({None: tick_clock.global_clock})
        )
        si = probe.ins.sync_info
        if si is not None and len(si.on_wait) > 1:
            waits = list(si.on_wait)
            probe.ins.sync_info = mybir.SyncInfo(
                on_wait=waits[:1], on_update=list(si.on_update)
            )
            for w in waits[1:]:
                extra = self.nc.sync.nop(nofuse=True)
                extra.ins.sync_info = mybir.SyncInfo(on_wait=[w], on_update=[])
        self.nc.sync.drain()
        self.nc.all_engine_barrier()
        popped = self.nc._tile_sem_poison_stack.pop()
        assert popped is self._sem_poison
        self.nc.clear_and_free_semaphores(list(self.sems.allocated().values()))
        self.nc.all_engine_barrier()

    tile.TileContext._drain_and_barrier = _drain_and_barrier
    tile.TileContext._drain_split_patch = True


def _split_excess_waits(nc, max_waits=1):
    """This walrus build has very few sync-wait slots per ISA instruction
    (a 2-wait TensorScalarPtr and a 3-wait Drain both fail codegen with
    'Too many sync wait commands').  Hoist all but one wait of every
    instruction into dedicated single-wait NOPs in front of it."""
    import concourse.mybir as mybir

    n = 0
    for f in nc.m.functions:
        for b in f.blocks:
            changed = False
            out = []
            for inst in b.instructions:
                si = inst.sync_info
                if si is not None and si.on_wait and len(si.on_wait) > max_waits:
                    changed = True
                    waits = list(si.on_wait)
                    for w in waits[max_waits:]:
                        n += 1
                        out.append(
                            mybir.InstNoOp(
                                name=f"{inst.name}_xw{n}",
                                sync_info=mybir.SyncInfo(on_wait=[w], on_update=[]),
                                bass_nofuse=True,
                                engine=inst.engine,
                            )
                        )
                    inst.sync_info = mybir.SyncInfo(
                        on_wait=waits[:max_waits], on_update=list(si.on_update)
                    )
                out.append(inst)
            if changed:
                b.instructions = out
    return n


def build_nc(dm=DM, nq=NQ, nkv=NKV, hpc=HPC, dh=DH):
    """Build the per-core Bass program (SPMD: same program, per-core data)."""
    import concourse.bass as bass
    import concourse.mybir as mybir
    import concourse.tile as tile

    _patch_tile_drain()

    f16 = mybir.dt.float16
    f32 = mybir.dt.float32
    f8 = mybir.dt.float8e4
    mdt = f8 if MASK_F8 else f16
    fpc = hpc * dh
    KT = dm // 128  # contraction tiles for projections
    FT = fpc // 128  # feature partition-tiles (2 heads per 128-row tile)
    KV = nkv // 128  # kv-token tiles
    QHW = nq // 2  # q-half width (1024)

    nc = bass.Bass(trn_type="TRN2")

    QT = nc.dram_tensor("QT", [dm, nq], f16, kind="ExternalInput")
    KTi = nc.dram_tensor("KTi", [dm, nkv], f16, kind="ExternalInput")
    VTi = nc.dram_tensor("VTi", [dm, nkv], f16, kind="ExternalInput")
    MT = nc.dram_tensor("MT", [nkv, nq], mdt, kind="ExternalInput")
    WQ = nc.dram_tensor("WQ", [dm, fpc], f16, kind="ExternalInput")
    WK = nc.dram_tensor("WK", [dm, fpc], f16, kind="ExternalInput")
    WV = nc.dram_tensor("WV", [dm, fpc], f16, kind="ExternalInput")
    WO = nc.dram_tensor("WO", [fpc, dm], f16, kind="ExternalInput")
    BQ = nc.dram_tensor("BQ", [fpc], f32, kind="ExternalInput")
    BK = nc.dram_tensor("BK", [fpc], f32, kind="ExternalInput")
    OUT = nc.dram_tensor("OUT", [nq, dm], f32, kind="ExternalOutput")

    with tile.TileContext(nc) as tc:
        with (
            tc.tile_pool(name="wpool", bufs=1) as wpool,
            tc.tile_pool(name="mpool", bufs=1) as mpool,
            tc.tile_pool(name="big", bufs=1) as bigpool,
            tc.tile_pool(name="qin", bufs=8) as qin,
            tc.tile_pool(name="kin", bufs=8) as kin,
            tc.tile_pool(name="vin", bufs=4) as vin,
            tc.tile_pool(name="pt", bufs=4) as ptpool,
            tc.tile_pool(name="pvsb", bufs=2) as pvpool,
            tc.tile_pool(name="rec", bufs=2) as recpool,
            tc.tile_pool(name="outp", bufs=3) as outpool,
            tc.tile_pool(name="psS", bufs=2, space="PSUM") as psS,
            tc.tile_pool(name="psV", bufs=2, space="PSUM") as psV,
            tc.tile_pool(name="dramp", bufs=8, space="DRAM") as dramp,
        ):
            # ---- weight + input DMAs, strictly in consumption order ----
            wq_sb = wpool.tile([128, KT, fpc], f16)
            for kt in range(KT):
                nc.sync.dma_start(
                    out=wq_sb[:, kt, :], in_=WQ[kt * 128 : (kt + 1) * 128, :]
                )
            bq_sb = wpool.tile([128, FT], f32)
            nc.sync.dma_start(out=bq_sb, in_=BQ[:].rearrange("(t p) -> p t", p=128))
            qts = []
            for kt in range(KT):
                xt = qin.tile([128, nq], f16, tag="qin")
                nc.sync.dma_start(out=xt, in_=QT[kt * 128 : (kt + 1) * 128, :])
                qts.append(xt)
            wk_sb = wpool.tile([128, KT, fpc], f16)
            for kt in range(KT):
                nc.sync.dma_start(
                    out=wk_sb[:, kt, :], in_=WK[kt * 128 : (kt + 1) * 128, :]
                )
            bk_sb = wpool.tile([128, FT], f32)
            nc.sync.dma_start(out=bk_sb, in_=BK[:].rearrange("(t p) -> p t", p=128))
            kts = []
            for kt in range(KT):
                xt = kin.tile([128, nkv], f16, tag="kin")
                nc.sync.dma_start(out=xt, in_=KTi[kt * 128 : (kt + 1) * 128, :])
                kts.append(xt)
            wv_sb = wpool.tile([128, KT, fpc], f16)
            nc.sync.dma_start(
                out=wv_sb, in_=WV[:].rearrange("(kt p) f -> p kt f", p=128)
            )
            # mask + V stream per kv tile (consumed in kv order by head 0)
            mt_sb = mpool.tile([128, KV, nq], mdt)
            vts = []
            for kv in range(KV):
                nc.sync.dma_start(
                    out=mt_sb[:, kv, :],
                    in_=MT[kv * 128 : (kv + 1) * 128, :],
                )
                vt = vin.tile([128, KT, 128], f16, tag="vin")
                nc.sync.dma_start(
                    out=vt,
                    in_=VTi[:, kv * 128 : (kv + 1) * 128].rearrange(
                        "(kt p) c -> p kt c", p=128
                    ),
                )
                vts.append(vt)
            wo_sb = wpool.tile([128, FT, dm], f16)
            nc.sync.dma_start(
                out=wo_sb, in_=WO[:].rearrange("(kt p) f -> p kt f", p=128)
            )

            # ---- persistent SBUF state ----
            # qTz: per-head q features zero-padded to full 128 contraction
            # rows.  Head h occupies rows (h%2)*64..(h%2)*64+64 (its natural
            # position in the ft-pair projection); the other 64 rows are
            # zero.  Scores then contract all 128 rows against the NATURAL
            # head-pair kT layout: the zero rows annihilate the other
            # head's contribution.  This keeps every matmul a full
            # 128-partition stream (measured 379ns/512col vs 630ns for
            # K=64 on this part).
            qTz = bigpool.tile([128, hpc, nq], f16)
            nc.vector.memset(qTz, 0.0)
            kT_sb = bigpool.tile([128, FT, nkv], f16)
            attnT_sb = bigpool.tile([128, FT, nq], f16)
            v_sb = bigpool.tile([128, KV, hpc, dh + 1], f16)
            nc.vector.memset(v_sb[:, :, :, dh : dh + 1], 1.0)

            # ---- helper: one q/k projection unit ([128, 512] psum) ----
            # For Q (per_head=True) the psum's two 64-row halves are written
            # to the two heads' slots in qTz (partition-aligned, no shift).
            def qk_proj_unit(w_sb, b_sb, xts, ft, c0, dest, per_head=False):
                ps = psS.tile([128, 512], f32, tag="ps", name=f"pj_{ft}_{c0}")
                for kt in range(KT):
                    nc.tensor.matmul(
                        ps,
                        w_sb[:, kt, ft * 128 : (ft + 1) * 128],
                        xts[kt][:, c0 : c0 + 512],
                        start=(kt == 0),
                        stop=(kt == KT - 1),
                    )
                if per_head:
                    for half in range(2):
                        h = 2 * ft + half
                        ro = half * 64
                        nc.vector.tensor_scalar_add(
                            out=dest[ro : ro + 64, h, c0 : c0 + 512],
                            in0=ps[ro : ro + 64, :],
                            scalar1=b_sb[ro : ro + 64, ft : ft + 1],
                        )
                else:
                    nc.vector.tensor_scalar_add(
                        out=dest[:, ft, c0 : c0 + 512],
                        in0=ps,
                        scalar1=b_sb[:, ft : ft + 1],
                    )

            # ---- helper: v projection for one kv tile ----
            def v_proj_unit(kv):
                ps = psS.tile([128, fpc], f32, tag="ps", name=f"vp_{kv}")
                for kt in range(KT):
                    nc.tensor.matmul(
                        ps,
                        vts[kv][:, kt, :],
                        wv_sb[:, kt, :],
                        start=(kt == 0),
                        stop=(kt == KT - 1),
                    )
                nc.vector.tensor_copy(
                    out=v_sb[:, kv, :, 0:dh],
                    in_=ps.rearrange("p (h d) -> p h d", d=dh),
                )

            # ---- helper: output projection for one token tile ----
            def out_proj_unit(tt):
                for nb in range(dm // 512):
                    ps = psS.tile(
                        [128, 512], f32, tag="ps", name=f"op_{tt}_{nb}"
                    )
                    for ft in range(FT):
                        nc.tensor.matmul(
                            ps,
                            attnT_sb[:, ft, tt * 128 : (tt + 1) * 128],
                            wo_sb[:, ft, nb * 512 : (nb + 1) * 512],
                            start=(ft == 0),
                            stop=(ft == FT - 1),
                        )
                    ob = outpool.tile([128, 512], f32, tag="outp")
                    nc.vector.tensor_copy(out=ob, in_=ps)
                    nc.sync.dma_start(
                        out=OUT[
                            tt * 128 : (tt + 1) * 128, nb * 512 : (nb + 1) * 512
                        ],
                        in_=ob,
                    )

            # ---- helper: attention for one (head, q-half, kv tile) ----
            def attn_iter(h, qh, kv, pv_ps):
                ft = h // 2
                ro = (h % 2) * 64
                q0 = qh * QHW
                sc = psS.tile([128, QHW], f32, tag="ps", name=f"sc_{h}_{qh}_{kv}")
                for half in range(QHW // 512):
                    nc.tensor.matmul(
                        sc[:, half * 512 : (half + 1) * 512],
                        kT_sb[:, ft, kv * 128 : (kv + 1) * 128],
                        qTz[:, h, q0 + half * 512 : q0 + (half + 1) * 512],
                        start=True,
                        stop=True,
                    )
                pt = ptpool.tile([128, QHW], f16, tag="pt")
                nc.scalar.activation(
                    out=pt,
                    in_=sc,
                    func=mybir.ActivationFunctionType.Exp,
                    scale=float(1.0 / np.sqrt(dh)),
                )
                nc.vector.tensor_mul(pt, pt, mt_sb[:, kv, q0 : q0 + QHW])
                for half in range(QHW // 512):
                    nc.tensor.matmul(
                        pv_ps[:, half * 512 : (half + 1) * 512],
                        v_sb[:, kv, h, :],
                        pt[:, half * 512 : (half + 1) * 512],
                        start=(kv == 0),
                        stop=(kv == KV - 1),
                    )

            # ---- helper: per-(head, q-half) epilogue ----
            def epilogue(h, qh, pv_ps):
                ft = h // 2
                ro = (h % 2) * 64
                q0 = qh * QHW
                pv_sb = pvpool.tile(
                    [dh + 1, QHW], f32, tag="pvsb", name=f"pvsb_{h}_{qh}"
                )
                nc.vector.tensor_copy(out=pv_sb, in_=pv_ps)
                # 1/denom as exp(-ln d) on the ACT engine (frees the DVE;
                # InstReciprocal measures 6.5us per call here).  Ln reads
                # the denominator row straight from PSUM so it runs in
                # parallel with the pv copy.
                lnd = recpool.tile(
                    [1, QHW], f32, tag="recf", name=f"recf_{h}_{qh}"
                )
                nc.scalar.activation(
                    out=lnd,
                    in_=pv_ps[dh : dh + 1, :],
                    func=mybir.ActivationFunctionType.Ln,
                )
                rec = recpool.tile([1, QHW], f16, tag="rec", name=f"rec_{h}_{qh}")
                with nc.allow_low_precision(reason="fp16 softmax denominators"):
                    nc.scalar.activation(
                        out=rec,
                        in_=lnd,
                        func=mybir.ActivationFunctionType.Exp,
                        scale=-1.0,
                    )
                rd = dramp.tile([QHW], f16, tag="rd", name=f"rd_{h}_{qh}")
                nc.sync.dma_start(out=rd, in_=rec)
                recb = recpool.tile(
                    [64, QHW], f16, tag="recb", name=f"recb_{h}_{qh}"
                )
                nc.sync.dma_start(
                    out=recb, in_=rd.unsqueeze(0).to_broadcast([64, QHW])
                )
                nc.vector.tensor_mul(
                    attnT_sb[ro : ro + 64, ft, q0 : q0 + QHW],
                    pv_sb[0:dh, :],
                    recb,
                )

            # ================= pipeline =================
            # prolog: ft0 q/k projections (heads 0,1)
            for c0 in range(0, QHW, 512):
                qk_proj_unit(wq_sb, bq_sb, qts, 0, c0, qTz, per_head=True)
            for c0 in range(0, nkv, 512):
                qk_proj_unit(wk_sb, bk_sb, kts, 0, c0, kT_sb)
            for c0 in range(QHW, nq, 512):
                qk_proj_unit(wq_sb, bq_sb, qts, 0, c0, qTz, per_head=True)

            # deferred PE work, one unit injected every other attention iter
            ft1_units = (
                [("q", wq_sb, bq_sb, qts, 1, c0, qTz) for c0 in range(0, QHW, 512)]
                + [("k", wk_sb, bk_sb, kts, 1, c0, kT_sb) for c0 in range(0, nkv, 512)]
                + [("q", wq_sb, bq_sb, qts, 1, c0, qTz) for c0 in range(QHW, nq, 512)]
            )

            for qh in range(2):
                for h in range(hpc):
                    pv_ps = psV.tile(
                        [dh + 1, QHW], f32, tag="pv", name=f"pv_{h}_{qh}"
                    )
                    if qh == 0 and h == 0:
                        v_proj_unit(0)
                    for kv in range(KV):
                        attn_iter(h, qh, kv, pv_ps)
                        if qh == 0 and h == 0:
                            if kv + 1 < KV:
                                v_proj_unit(kv + 1)
                        elif qh == 0 and h == 1 and kv % 2 == 0 and kv // 2 < len(
                            ft1_units
                        ):
                            u = ft1_units[kv // 2]
                            qk_proj_unit(
                                u[1], u[2], u[3], u[4], u[5], u[6],
                                per_head=(u[0] == "q"),
                            )
                        elif qh == 0 and h == 2 and 8 + kv // 2 < len(
                            ft1_units
                        ) and kv % 2 == 0:
                            u = ft1_units[8 + kv // 2]
                            qk_proj_unit(
                                u[1], u[2], u[3], u[4], u[5], u[6],
                                per_head=(u[0] == "q"),
                            )
                        elif qh == 1 and h == 0 and 4 <= kv < 12:
                            # output projection for q-half 0 (tok tiles 0..7);
                            # delayed to kv=4 so the h3/qh0 epilogue chain
                            # (pv copy -> ln -> exp -> bcast -> norm) finishes
                            out_proj_unit(kv - 4)
                    epilogue(h, qh, pv_ps)

            # tail: output projection for q-half 1
            for tt in range(nq // 256, nq // 128):
                out_proj_unit(tt)

    if not int(os.environ.get("KERNEL_NO_WAITSPLIT", "0")):
        _split_excess_waits(nc)
    return nc


def _get_nc():
    if "nc" not in _cache:
        _cache["nc"] = build_nc()
    return _cache["nc"]


def kernel(Q, K, V, mask, W_q, b_q, W_k, b_k, W_v, b_v, W_o, b_o):
    global last_results
    from concourse.bass_utils import run_bass_kernel_spmd

    nc = _get_nc()
    mdt = _mask_np_dtype()

    # host-side shard prep (layout massaging only)
    qt = [np.ascontiguousarray(Q[b].T).astype(_F16) for b in range(B)]
    kt = [np.ascontiguousarray(K[b].T).astype(_F16) for b in range(B)]
    vt = [np.ascontiguousarray(V[b].T).astype(_F16) for b in range(B)]
    mt = [np.ascontiguousarray(mask[b, 0].T).astype(mdt) for b in range(B)]
    wq = [
        np.ascontiguousarray(W_q[:, g * FPC : (g + 1) * FPC]).astype(_F16)
        for g in range(CORES_PER_BATCH)
    ]
    wk = [
        np.ascontiguousarray(W_k[:, g * FPC : (g + 1) * FPC]).astype(_F16)
        for g in range(CORES_PER_BATCH)
    ]
    wv = [
        np.ascontiguousarray(W_v[:, g * FPC : (g + 1) * FPC]).astype(_F16)
        for g in range(CORES_PER_BATCH)
    ]
    wo = [
        np.ascontiguousarray(W_o[g * FPC : (g + 1) * FPC, :]).astype(_F16)
        for g in range(CORES_PER_BATCH)
    ]
    bq = [
        np.ascontiguousarray(b_q[g * FPC : (g + 1) * FPC]).astype(np.float32)
        for g in range(CORES_PER_BATCH)
    ]
    bk = [
        np.ascontiguousarray(b_k[g * FPC : (g + 1) * FPC]).astype(np.float32)
        for g in range(CORES_PER_BATCH)
    ]

    in_maps = []
    for c in range(N_CORES):
        b, g = c // CORES_PER_BATCH, c % CORES_PER_BATCH
        in_maps.append(
            {
                "QT": qt[b],
                "KTi": kt[b],
                "VTi": vt[b],
                "MT": mt[b],
                "WQ": wq[g],
                "WK": wk[g],
                "WV": wv[g],
                "WO": wo[g],
                "BQ": bq[g],
                "BK": bk[g],
            }
        )

    trace = bool(int(os.environ.get("KERNEL_TRACE", "0")))
    res = run_bass_kernel_spmd(
        nc, in_maps, core_ids=list(range(N_CORES)), trace=trace
    )
    last_results = res

    out = np.zeros((B, NQ, DM), np.float32)
    for c in range(N_CORES):
        out[c // CORES_PER_BATCH] += res.results[c]["OUT"]
    # v-bias contributes b_v @ W_o to every row post-softmax; b_o is additive.
    out += (
        np.asarray(b_v, np.float32) @ np.asarray(W_o, np.float32)
        + np.asarray(b_o, np.float32)
    )
    return out


# revision 16
# speedup vs baseline: 1.0536x; 1.0536x over previous
"""MultiHeadAttention forward on 8 Trainium2 NeuronCores.

Sharding: core c handles batch (c // 4) and a block of 4 heads
(heads 4*(c%4) .. 4*(c%4)+4), i.e. 256 of the 1024 inner features.
Each core computes its batch's full attention for its heads plus the
partial output projection (rows of W_o for its heads); the host sums
the 4 per-batch partials (the W_o all-reduce) and adds the bias terms.

Pipeline (v3): q-half outer (1024 cols), then head, then kv tile.
Per (head, kv): PE scores (K=128 via zero-padded per-head q copies) ->
ACT exp -> DVE mask-mult (fp8 mask) -> PE PV accumulate (ones-column
denominator).  Projections and the output projection are interleaved
into the PE stream.  PSUM: 2 x scores[128,1024] + 2 x pv[65,1024] =
8 banks exactly.
"""

import os

import numpy as np

B = 2
NQ = 2048
NKV = 2048
DM = 1024
H = 16
DH = 64
N_CORES = 8
CORES_PER_BATCH = N_CORES // B  # 4
HPC = H // CORES_PER_BATCH  # 4 heads per core
FPC = HPC * DH  # 256 features per core

_F16 = np.float16

_cache = {}
last_results = None  # stash of BassKernelResults for test harnesses

MASK_F8 = not int(os.environ.get("KERNEL_MASK_F16", "0"))


def _mask_np_dtype():
    if not MASK_F8:
        return np.float16
    import ml_dtypes

    return ml_dtypes.float8_e4m3


def _patch_tile_drain():
    """Split the TileContext tail-drain waits: this walrus build rejects
    Drain instructions carrying more than one sync wait."""
    import concourse.mybir as mybir
    import concourse.tile as tile
    from concourse.vector_clock import ScopedClock

    if getattr(tile.TileContext, "_drain_split_patch", False):
        return

    def _drain_and_barrier(self, tick_clock, wait_clock):
        probe = self.nc.sync.nop(nofuse=True)
        wait_clock.add_sem_waits(
            probe.ins, ScopedClock({None: tick_clock.global_clock})
        )
        si = probe.ins.sync_info
        if si is not None and len(si.on_wait) > 1:
            waits = list(si.on_wait)
            probe.ins.sync_info = mybir.SyncInfo(
                on_wait=waits[:1], on_update=list(si.on_update)
            )
            for w in waits[1:]:
                extra = self.nc.sync.nop(nofuse=True)
                extra.ins.sync_info = mybir.SyncInfo(on_wait=[w], on_update=[])
        self.nc.sync.drain()
        self.nc.all_engine_barrier()
        popped = self.nc._tile_sem_poison_stack.pop()
        assert popped is self._sem_poison
        self.nc.clear_and_free_semaphores(list(self.sems.allocated().values()))
        self.nc.all_engine_barrier()

    tile.TileContext._drain_and_barrier = _drain_and_barrier
    tile.TileContext._drain_split_patch = True


def _split_excess_waits(nc, max_waits=1):
    """This walrus build has very few sync-wait slots per ISA instruction
    (a 2-wait TensorScalarPtr and a 3-wait Drain both fail codegen with
    'Too many sync wait commands').  Hoist all but one wait of every
    instruction into dedicated single-wait NOPs in front of it."""
    import concourse.mybir as mybir

    n = 0
    for f in nc.m.functions:
        for b in f.blocks:
            changed = False
            out = []
            for inst in b.instructions:
                si = inst.sync_info
                if si is not None and si.on_wait and len(si.on_wait) > max_waits:
                    changed = True
                    waits = list(si.on_wait)
                    for w in waits[max_waits:]:
                        n += 1
                        out.append(
                            mybir.InstNoOp(
                                name=f"{inst.name}_xw{n}",
                                sync_info=mybir.SyncInfo(on_wait=[w], on_update=[]),
                                bass_nofuse=True,
                                engine=inst.engine,
                            )
                        )
                    inst.sync_info = mybir.SyncInfo(
                        on_wait=waits[:max_waits], on_update=list(si.on_update)
                    )
                out.append(inst)
            if changed:
                b.instructions = out
    return n


def build_nc(dm=DM, nq=NQ, nkv=NKV, hpc=HPC, dh=DH):
    """Build the per-core Bass program (SPMD: same program, per-core data)."""
    import concourse.bass as bass
    import concourse.mybir as mybir
    import concourse.tile as tile

    _patch_tile_drain()

    f16 = mybir.dt.float16
    f32 = mybir.dt.float32
    f8 = mybir.dt.float8e4
    mdt = f8 if MASK_F8 else f16
    fpc = hpc * dh
    KT = dm // 128  # contraction tiles for projections
    FT = fpc // 128  # feature partition-tiles (2 heads per 128-row tile)
    KV = nkv // 128  # kv-token tiles
    QHW = nq // 2  # q-half width (1024)

    nc = bass.Bass(trn_type="TRN2")

    QT = nc.dram_tensor("QT", [dm, nq], f16, kind="ExternalInput")
    KTi = nc.dram_tensor("KTi", [dm, nkv], f16, kind="ExternalInput")
    VTi = nc.dram_tensor("VTi", [dm, nkv], f16, kind="ExternalInput")
    MT = nc.dram_tensor("MT", [nkv, nq], mdt, kind="ExternalInput")
    WQ = nc.dram_tensor("WQ", [dm, fpc], f16, kind="ExternalInput")
    WK = nc.dram_tensor("WK", [dm, fpc], f16, kind="ExternalInput")
    WV = nc.dram_tensor("WV", [dm, fpc], f16, kind="ExternalInput")
    WO = nc.dram_tensor("WO", [fpc, dm], f16, kind="ExternalInput")
    BQ = nc.dram_tensor("BQ", [fpc], f32, kind="ExternalInput")
    BK = nc.dram_tensor("BK", [fpc], f32, kind="ExternalInput")
    OUT = nc.dram_tensor("OUT", [nq, dm], f32, kind="ExternalOutput")

    with tile.TileContext(nc) as tc:
        with (
            tc.tile_pool(name="wpool", bufs=1) as wpool,
            tc.tile_pool(name="mpool", bufs=1) as mpool,
            tc.tile_pool(name="big", bufs=1) as bigpool,
            tc.tile_pool(name="qin", bufs=8) as qin,
            tc.tile_pool(name="kin", bufs=8) as kin,
            tc.tile_pool(name="vin", bufs=4) as vin,
            tc.tile_pool(name="pt", bufs=4) as ptpool,
            tc.tile_pool(name="pvsb", bufs=2) as pvpool,
            tc.tile_pool(name="rec", bufs=2) as recpool,
            tc.tile_pool(name="outp", bufs=3) as outpool,
            tc.tile_pool(name="psS", bufs=2, space="PSUM") as psS,
            tc.tile_pool(name="psV", bufs=2, space="PSUM") as psV,
            tc.tile_pool(name="dramp", bufs=8, space="DRAM") as dramp,
        ):
            # ---- weight + input DMAs, strictly in consumption order ----
            wq_sb = wpool.tile([128, KT, fpc], f16)
            nc.sync.dma_start(
                out=wq_sb, in_=WQ[:].rearrange("(kt p) f -> p kt f", p=128)
            )
            bq_sb = wpool.tile([128, FT], f32)
            nc.sync.dma_start(out=bq_sb, in_=BQ[:].rearrange("(t p) -> p t", p=128))
            qts = []
            for kt in range(KT):
                xt = qin.tile([128, nq], f16, tag="qin")
                nc.sync.dma_start(out=xt, in_=QT[kt * 128 : (kt + 1) * 128, :])
                qts.append(xt)
            wk_sb = wpool.tile([128, KT, fpc], f16)
            nc.sync.dma_start(
                out=wk_sb, in_=WK[:].rearrange("(kt p) f -> p kt f", p=128)
            )
            bk_sb = wpool.tile([128, FT], f32)
            nc.sync.dma_start(out=bk_sb, in_=BK[:].rearrange("(t p) -> p t", p=128))
            kts = []
            for kt in range(KT):
                xt = kin.tile([128, nkv], f16, tag="kin")
                nc.sync.dma_start(out=xt, in_=KTi[kt * 128 : (kt + 1) * 128, :])
                kts.append(xt)
            wv_sb = wpool.tile([128, KT, fpc], f16)
            nc.sync.dma_start(
                out=wv_sb, in_=WV[:].rearrange("(kt p) f -> p kt f", p=128)
            )
            # mask + V stream per kv tile (consumed in kv order by head 0)
            mt_sb = mpool.tile([128, KV, nq], mdt)
            vts = []
            for kv in range(KV):
                nc.sync.dma_start(
                    out=mt_sb[:, kv, :],
                    in_=MT[kv * 128 : (kv + 1) * 128, :],
                )
                vt = vin.tile([128, KT, 128], f16, tag="vin")
                nc.sync.dma_start(
                    out=vt,
                    in_=VTi[:, kv * 128 : (kv + 1) * 128].rearrange(
                        "(kt p) c -> p kt c", p=128
                    ),
                )
                vts.append(vt)
            wo_sb = wpool.tile([128, FT, dm], f16)
            nc.sync.dma_start(
                out=wo_sb, in_=WO[:].rearrange("(kt p) f -> p kt f", p=128)
            )

            # ---- persistent SBUF state ----
            # qTz: per-head q features zero-padded to full 128 contraction
            # rows.  Head h occupies rows (h%2)*64..(h%2)*64+64 (its natural
            # position in the ft-pair projection); the other 64 rows are
            # zero.  Scores contract all 128 rows against the NATURAL
            # head-pair kT layout: the zero rows annihilate the other
            # head's contribution.  This keeps every matmul a full
            # 128-partition stream (measured 379ns/512col vs 630ns for
            # K=64 on this part).
            qTz = bigpool.tile([128, hpc, nq], f16)
            nc.vector.memset(qTz, 0.0)
            kT_sb = bigpool.tile([128, FT, nkv], f16)
            attnT_sb = bigpool.tile([128, FT, nq], f16)
            v_sb = bigpool.tile([128, KV, hpc, dh + 1], f16)
            nc.vector.memset(v_sb[:, :, :, dh : dh + 1], 1.0)

            # ---- helper: one q/k projection unit ([128, 512] psum) ----
            # For Q (per_head=True) the psum's two 64-row halves are written
            # to the two heads' slots in qTz (partition-aligned, no shift).
            def qk_proj_unit(w_sb, b_sb, xts, ft, c0, dest, per_head=False):
                ps = psS.tile([128, 512], f32, tag="ps", name=f"pj_{ft}_{c0}")
                for kt in range(KT):
                    nc.tensor.matmul(
                        ps,
                        w_sb[:, kt, ft * 128 : (ft + 1) * 128],
                        xts[kt][:, c0 : c0 + 512],
                        start=(kt == 0),
                        stop=(kt == KT - 1),
                    )
                if per_head:
                    for half in range(2):
                        h = 2 * ft + half
                        ro = half * 64
                        nc.vector.tensor_scalar_add(
                            out=dest[ro : ro + 64, h, c0 : c0 + 512],
                            in0=ps[ro : ro + 64, :],
                            scalar1=b_sb[ro : ro + 64, ft : ft + 1],
                        )
                else:
                    nc.vector.tensor_scalar_add(
                        out=dest[:, ft, c0 : c0 + 512],
                        in0=ps,
                        scalar1=b_sb[:, ft : ft + 1],
                    )

            # ---- helper: v projection for one kv tile ----
            def v_proj_unit(kv):
                ps = psS.tile([128, fpc], f32, tag="ps", name=f"vp_{kv}")
                for kt in range(KT):
                    nc.tensor.matmul(
                        ps,
                        vts[kv][:, kt, :],
                        wv_sb[:, kt, :],
                        start=(kt == 0),
                        stop=(kt == KT - 1),
                    )
                nc.vector.tensor_copy(
                    out=v_sb[:, kv, :, 0:dh],
                    in_=ps.rearrange("p (h d) -> p h d", d=dh),
                )

            # ---- helper: output projection for one token tile ----
            def out_proj_unit(tt):
                for nb in range(dm // 512):
                    ps = psS.tile(
                        [128, 512], f32, tag="ps", name=f"op_{tt}_{nb}"
                    )
                    for ft in range(FT):
                        nc.tensor.matmul(
                            ps,
                            attnT_sb[:, ft, tt * 128 : (tt + 1) * 128],
                            wo_sb[:, ft, nb * 512 : (nb + 1) * 512],
                            start=(ft == 0),
                            stop=(ft == FT - 1),
                        )
                    ob = outpool.tile([128, 512], f32, tag="outp")
                    nc.vector.tensor_copy(out=ob, in_=ps)
                    nc.sync.dma_start(
                        out=OUT[
                            tt * 128 : (tt + 1) * 128, nb * 512 : (nb + 1) * 512
                        ],
                        in_=ob,
                    )

            # ---- helper: attention for one (head, q-half, kv tile) ----
            def attn_iter(h, qh, kv, pv_ps):
                ft = h // 2
                q0 = qh * QHW
                sc = psS.tile([128, QHW], f32, tag="ps", name=f"sc_{h}_{qh}_{kv}")
                for half in range(QHW // 512):
                    nc.tensor.matmul(
                        sc[:, half * 512 : (half + 1) * 512],
                        kT_sb[:, ft, kv * 128 : (kv + 1) * 128],
                        qTz[:, h, q0 + half * 512 : q0 + (half + 1) * 512],
                        start=True,
                        stop=True,
                    )
                pt = ptpool.tile([128, QHW], f16, tag="pt")
                nc.scalar.activation(
                    out=pt,
                    in_=sc,
                    func=mybir.ActivationFunctionType.Exp,
                    scale=float(1.0 / np.sqrt(dh)),
                )
                nc.vector.tensor_mul(pt, pt, mt_sb[:, kv, q0 : q0 + QHW])
                for half in range(QHW // 512):
                    nc.tensor.matmul(
                        pv_ps[:, half * 512 : (half + 1) * 512],
                        v_sb[:, kv, h, :],
                        pt[:, half * 512 : (half + 1) * 512],
                        start=(kv == 0),
                        stop=(kv == KV - 1),
                    )

            # ---- helper: per-(head, q-half) epilogue ----
            def epilogue(h, qh, pv_ps):
                ft = h // 2
                ro = (h % 2) * 64
                q0 = qh * QHW
                pv_sb = pvpool.tile(
                    [dh + 1, QHW], f32, tag="pvsb", name=f"pvsb_{h}_{qh}"
                )
                nc.vector.tensor_copy(out=pv_sb, in_=pv_ps)
                # 1/denom as exp(-ln d) on the ACT engine (frees the DVE;
                # InstReciprocal measures 6.5us per call here).
                lnd = recpool.tile(
                    [1, QHW], f32, tag="recf", name=f"recf_{h}_{qh}"
                )
                nc.scalar.activation(
                    out=lnd,
                    in_=pv_sb[dh : dh + 1, :],
                    func=mybir.ActivationFunctionType.Ln,
                )
                rec = recpool.tile([1, QHW], f16, tag="rec", name=f"rec_{h}_{qh}")
                with nc.allow_low_precision(reason="fp16 softmax denominators"):
                    nc.scalar.activation(
                        out=rec,
                        in_=lnd,
                        func=mybir.ActivationFunctionType.Exp,
                        scale=-1.0,
                    )
                rd = dramp.tile([QHW], f16, tag="rd", name=f"rd_{h}_{qh}")
                nc.sync.dma_start(out=rd, in_=rec)
                recb = recpool.tile(
                    [64, QHW], f16, tag="recb", name=f"recb_{h}_{qh}"
                )
                nc.sync.dma_start(
                    out=recb, in_=rd.unsqueeze(0).to_broadcast([64, QHW])
                )
                nc.vector.tensor_mul(
                    attnT_sb[ro : ro + 64, ft, q0 : q0 + QHW],
                    pv_sb[0:dh, :],
                    recb,
                )

            # ================= pipeline =================
            # prolog: ft0 q/k projections (heads 0,1)
            for c0 in range(0, QHW, 512):
                qk_proj_unit(wq_sb, bq_sb, qts, 0, c0, qTz, per_head=True)
            for c0 in range(0, nkv, 512):
                qk_proj_unit(wk_sb, bk_sb, kts, 0, c0, kT_sb)
            for c0 in range(QHW, nq, 512):
                qk_proj_unit(wq_sb, bq_sb, qts, 0, c0, qTz, per_head=True)

            # deferred PE work, one unit injected every other attention iter
            ft1_units = (
                [("q", wq_sb, bq_sb, qts, 1, c0, qTz) for c0 in range(0, QHW, 512)]
                + [("k", wk_sb, bk_sb, kts, 1, c0, kT_sb) for c0 in range(0, nkv, 512)]
                + [("q", wq_sb, bq_sb, qts, 1, c0, qTz) for c0 in range(QHW, nq, 512)]
            )

            for qh in range(2):
                for h in range(hpc):
                    pv_ps = psV.tile(
                        [dh + 1, QHW], f32, tag="pv", name=f"pv_{h}_{qh}"
                    )
                    if qh == 0 and h == 0:
                        v_proj_unit(0)
                    for kv in range(KV):
                        attn_iter(h, qh, kv, pv_ps)
                        if qh == 0 and h == 0:
                            if kv + 1 < KV:
                                v_proj_unit(kv + 1)
                        elif qh == 0 and h == 1 and kv % 2 == 0 and kv // 2 < len(
                            ft1_units
                        ):
                            u = ft1_units[kv // 2]
                            qk_proj_unit(
                                u[1], u[2], u[3], u[4], u[5], u[6],
                                per_head=(u[0] == "q"),
                            )
                        elif qh == 0 and h == 2 and 8 + kv // 2 < len(
                            ft1_units
                        ) and kv % 2 == 0:
                            u = ft1_units[8 + kv // 2]
                            qk_proj_unit(
                                u[1], u[2], u[3], u[4], u[5], u[6],
                                per_head=(u[0] == "q"),
                            )
                        elif qh == 1 and h == 0 and 4 <= kv < 12:
                            # output projection for q-half 0 (tok tiles 0..7);
                            # delayed to kv=4 so the h3/qh0 epilogue chain
                            # (pv copy -> ln -> exp -> bcast -> norm) finishes
                            out_proj_unit(kv - 4)
                    epilogue(h, qh, pv_ps)

            # tail: output projection for q-half 1
            for tt in range(nq // 256, nq // 128):
                out_proj_unit(tt)

    if not int(os.environ.get("KERNEL_NO_WAITSPLIT", "0")):
        _split_excess_waits(nc)
    return nc


def _get_nc():
    if "nc" not in _cache:
        _cache["nc"] = build_nc()
    return _cache["nc"]


def kernel(Q, K, V, mask, W_q, b_q, W_k, b_k, W_v, b_v, W_o, b_o):
    global last_results
    from concourse.bass_utils import run_bass_kernel_spmd

    nc = _get_nc()
    mdt = _mask_np_dtype()

    # host-side shard prep (layout massaging only)
    qt = [np.ascontiguousarray(Q[b].T).astype(_F16) for b in range(B)]
    kt = [np.ascontiguousarray(K[b].T).astype(_F16) for b in range(B)]
    vt = [np.ascontiguousarray(V[b].T).astype(_F16) for b in range(B)]
    mt = [np.ascontiguousarray(mask[b, 0].T).astype(mdt) for b in range(B)]
    wq = [
        np.ascontiguousarray(W_q[:, g * FPC : (g + 1) * FPC]).astype(_F16)
        for g in range(CORES_PER_BATCH)
    ]
    wk = [
        np.ascontiguousarray(W_k[:, g * FPC : (g + 1) * FPC]).astype(_F16)
        for g in range(CORES_PER_BATCH)
    ]
    wv = [
        np.ascontiguousarray(W_v[:, g * FPC : (g + 1) * FPC]).astype(_F16)
        for g in range(CORES_PER_BATCH)
    ]
    wo = [
        np.ascontiguousarray(W_o[g * FPC : (g + 1) * FPC, :]).astype(_F16)
        for g in range(CORES_PER_BATCH)
    ]
    bq = [
        np.ascontiguousarray(b_q[g * FPC : (g + 1) * FPC]).astype(np.float32)
        for g in range(CORES_PER_BATCH)
    ]
    bk = [
        np.ascontiguousarray(b_k[g * FPC : (g + 1) * FPC]).astype(np.float32)
        for g in range(CORES_PER_BATCH)
    ]

    in_maps = []
    for c in range(N_CORES):
        b, g = c // CORES_PER_BATCH, c % CORES_PER_BATCH
        in_maps.append(
            {
                "QT": qt[b],
                "KTi": kt[b],
                "VTi": vt[b],
                "MT": mt[b],
                "WQ": wq[g],
                "WK": wk[g],
                "WV": wv[g],
                "WO": wo[g],
                "BQ": bq[g],
                "BK": bk[g],
            }
        )

    trace = bool(int(os.environ.get("KERNEL_TRACE", "0")))
    res = run_bass_kernel_spmd(
        nc, in_maps, core_ids=list(range(N_CORES)), trace=trace
    )
    last_results = res

    out = np.zeros((B, NQ, DM), np.float32)
    for c in range(N_CORES):
        out[c // CORES_PER_BATCH] += res.results[c]["OUT"]
    # v-bias contributes b_v @ W_o to every row post-softmax; b_o is additive.
    out += (
        np.asarray(b_v, np.float32) @ np.asarray(W_o, np.float32)
        + np.asarray(b_o, np.float32)
    )
    return out
